# revision 1
# baseline (speedup 1.0000x reference)
"""Bahdanau attention TRN2 Bass kernel.

kernel(**inputs) takes the FULL inputs (as produced by setup_inputs()):
    dec_hidden [32, 1024] f32, enc_outputs [32, 2048, 2048] f32,
    W_s [1024, 1024] f32, W_h [1024, 2048] f32, v [1024] f32
and returns (ctx [32, 2048] f32, attn [32, 2048] f32), matching

    s      = dec_hidden @ W_s.T
    h      = enc_outputs @ W_h.T
    scores = einsum('bld,d->bl', tanh(s[:,None,:] + h), v)
    attn   = softmax(scores, axis=1)
    ctx    = einsum('bl,ble->be', attn, enc_outputs)

Distribution: data-parallel over batch, 4 batch elements per NeuronCore on
8 cores, no collectives.

Design notes:
  - enc is fed from the host in BOTH layouts the kernel needs -- e-major
    (encT, moving operand of the projection matmul) and l-major (encN,
    moving operand of the context matmul) -- pre-tiled so every DMA is a
    2D copy with 16KB-contiguous per-partition runs.  No on-chip
    transposes of enc at all.
  - All big matmul operands are bf16 (halves DMA + SBUF); accumulation
    is fp32 in PSUM.
  - hT[d, l] is produced with d on partitions so the s-projection folds
    into the ACT engine's per-partition bias and tanh(h + s) is a single
    ACT pass over the matmul PSUM output.
  - softmax runs without max-subtraction: |scores| <= sum|v| ~ 26, so
    exp stays comfortably inside fp32 range and the whole chunk pipeline
    is one exp (with accumulated Z) + one tiny transpose for the context
    weights.
  - The v-dot and the per-chunk context reduction are reformulated as
    per-partition weighted sums on the DVE (scalar_tensor_tensor FMA
    with a per-partition scalar), so the PE -- the bottleneck engine --
    only does a single ones-vector cross-partition reduction matmul for
    each instead of M=1 chains (saves ~65us of PE time per core).
  - ctx partials accumulate across the batch in four PSUM banks;
    everything is normalized by 1/Z once at the end of each batch
    element.  NOTE: start=True clears has_written for the WHOLE PSUM
    bank, so accumulation chains must never share a bank.
"""

import json as _json
from contextlib import ExitStack

import numpy as np
import ml_dtypes

_BF16 = ml_dtypes.bfloat16

_B, _L, _D, _E = 32, 2048, 1024, 2048
_NCORES = 8
_P = 128
_LCHUNK = 512


# ----------------------------------------------------------------------------
# Workaround: this walrus build rejects instructions carrying more than one
# semaphore wait ("Too many sync wait commands").  Split extra waits onto
# preceding same-engine NoOps at BIR-serialization time.
# ----------------------------------------------------------------------------
_ws_counter = [0]


def _split_instruction_waits(inst, max_waits=1):
    waits = inst.get("sync_info", {}).get("on_wait") or []
    if len(waits) <= max_waits:
        return [inst]
    out = []
    extra = waits[:-max_waits]
    inst["sync_info"]["on_wait"] = waits[-max_waits:]
    for i in range(0, len(extra), max_waits):
        _ws_counter[0] += 1
        out.append({
            "debug": inst.get("debug", 0),
            "engine": inst["engine"],
            "ins": [],
            "name": f"I-ws{_ws_counter[0]}",
            "opcode": "NoOp",
            "outs": [],
            "sync_info": {"on_update": [], "on_wait": extra[i:i + max_waits]},
        })
    out.append(inst)
    return out


def _walk_split(obj):
    if isinstance(obj, dict):
        for key, val in obj.items():
            if key == "instructions" and isinstance(val, list):
                new = []
                for inst in val:
                    if isinstance(inst, dict) and "sync_info" in inst:
                        new.extend(_split_instruction_waits(inst))
                    else:
                        _walk_split(inst)
                        new.append(inst)
                obj[key] = new
            else:
                _walk_split(val)
    elif isinstance(obj, list):
        for item in obj:
            _walk_split(item)


def _install_waitsplit():
    import concourse.bass as bass
    if getattr(bass.Bass, "_waitsplit_installed", False):
        return
    orig = bass.Bass.to_json_bytes

    def to_json_bytes(self, *a, **kw):
        d = _json.loads(orig(self, *a, **kw))
        _walk_split(d)
        return _json.dumps(d).encode()

    bass.Bass.to_json_bytes = to_json_bytes
    bass.Bass._waitsplit_installed = True


# ----------------------------------------------------------------------------
# Kernel builder
# ----------------------------------------------------------------------------

def _build(Bc=4, L=_L, D=_D, E=_E, LCHUNK=_LCHUNK, reps=1, debug_outs=False):
    import concourse.bass as bass
    import concourse.mybir as mybir
    import concourse.tile as tile
    from concourse.masks import make_identity

    F32 = mybir.dt.float32
    F32R = mybir.dt.float32r
    BF16 = mybir.dt.bfloat16
    AF = mybir.ActivationFunctionType

    P = _P
    DT, ET = D // P, E // P          # 8, 16
    NCH = L // LCHUNK                # 4
    LCT = LCHUNK // P                # 4
    NE = E // 512                    # 4
    assert D % P == 0 and E % P == 0 and L % LCHUNK == 0 and LCHUNK % P == 0

    nc = bass.Bass("TRN2", target_bir_lowering=False, debug=False)
    # host-pretiled inputs (see prepare_in_maps for the exact layouts)
    encN = nc.dram_tensor("encN", [Bc, NCH, P, LCT * E], BF16,
                          kind="ExternalInput").ap()
    encT = nc.dram_tensor("encT", [Bc, NCH, P, ET * LCHUNK], BF16,
                          kind="ExternalInput").ap()
    whT = nc.dram_tensor("whT", [P, ET * D], BF16, kind="ExternalInput").ap()
    wsT = nc.dram_tensor("wsT", [P, DT * D], BF16, kind="ExternalInput").ap()
    decT = nc.dram_tensor("decT", [P, DT * Bc], BF16, kind="ExternalInput").ap()
    vT = nc.dram_tensor("vT", [P, DT], BF16, kind="ExternalInput").ap()
    ctx_o = nc.dram_tensor("ctx", [Bc, E], F32, kind="ExternalOutput").ap()
    attn_o = nc.dram_tensor("attn", [Bc, L], F32, kind="ExternalOutput").ap()
    if debug_outs:
        sT_dbg = nc.dram_tensor("sT_dbg", [P, DT * Bc], F32,
                                kind="ExternalOutput").ap()
        tj_dbg = nc.dram_tensor("tj_dbg", [P, LCHUNK], F32,
                                kind="ExternalOutput").ap()
        sc_dbg = nc.dram_tensor("sc_dbg", [Bc, L], F32,
                                kind="ExternalOutput").ap()

    with tile.TileContext(nc) as tc:
        with ExitStack() as es:
            const_p = es.enter_context(tc.tile_pool(name="const", bufs=1))
            w_p = es.enter_context(tc.tile_pool(name="wp", bufs=1))
            encT_p = es.enter_context(tc.tile_pool(name="encTp", bufs=2))
            encN_p = es.enter_context(tc.tile_pool(name="encNp", bufs=3))
            tj_p = es.enter_context(tc.tile_pool(name="tjp", bufs=4))
            uv_p = es.enter_context(tc.tile_pool(name="uvp", bufs=2))
            rows_p = es.enter_context(tc.tile_pool(name="rows", bufs=1))

            ident = const_p.tile([P, P], F32)
            make_identity(nc, ident[:])
            ones_raw = const_p.tile([P, 1], F32, tag="ones_raw",
                                    name="ones_raw")
            nc.vector.memset(ones_raw[:], 1.0)
            ones = const_p.tile([P, 1], F32, tag="ones", name="ones")
            nc.vector.tensor_copy(out=ones[:].bitcast(F32R), in_=ones_raw[:])

            state = {}

            def emit_load(b, c, parts="TN"):
                if "T" in parts:
                    eT = encT_p.tile([P, ET * LCHUNK], BF16, tag="encT",
                                     name=f"encT{b}_{c}")
                    nc.sync.dma_start(eT[:], encT[b, c])
                else:
                    eT = state[(b, c)][0]
                if "N" in parts:
                    eN = encN_p.tile([P, LCT * E], BF16, tag="encN",
                                     name=f"encN{b}_{c}")
                    nc.sync.dma_start(eN[:], encN[b, c])
                else:
                    eN = None
                state[(b, c)] = (eT, eN)

            # Startup critical path: chunk (0,0)'s encT on the sync ring;
            # the setup tensors go on the ACT ring in parallel (s inputs
            # first, then whT), and encN(0,0) -- not needed until the first
            # ctx pass -- last.
            emit_load(0, 0, parts="T")

            # ---- s inputs + resident weights (ACT HWDGE ring) ----
            wst_sb = w_p.tile([P, DT * D], BF16, tag="wst", name="wst_sb")
            nc.scalar.dma_start(wst_sb[:], wsT[:, :])
            dec_sb = w_p.tile([P, DT * Bc], BF16, tag="dec_sb", name="dec_sb")
            nc.scalar.dma_start(dec_sb[:], decT[:, :])
            v_sb = w_p.tile([P, DT], BF16, tag="v_sb", name="v_sb")
            nc.scalar.dma_start(v_sb[:], vT[:, :])
            v32 = w_p.tile([P, DT], F32, tag="v32", name="v32")
            nc.vector.tensor_copy(out=v32[:], in_=v_sb[:])
            whT_sb = w_p.tile([P, ET * D], BF16, tag="whT", name="whT_sb")
            quarter = ET * D // 4
            for q in range(4):
                nc.scalar.dma_start(whT_sb[:, q * quarter:(q + 1) * quarter],
                                    whT[:, q * quarter:(q + 1) * quarter])

            # ---- s-projection: sT[:, j*Bc + b] = s_b[j*128 + p] ----
            # NOTE: start=True clears has_written for the WHOLE PSUM bank, so
            # each of the 8 accumulation chains needs its own bank.  The
            # setup pool closes before the main PSUM pools open.
            sT = w_p.tile([P, DT * Bc], F32, tag="sT", name="sT_sb")
            with tc.tile_pool(name="ps_setup", bufs=1, space="PSUM") as ps_set:
                ps_s = [ps_set.tile([P, Bc], F32, tag=f"s{J}", name=f"ps_s{J}")
                        for J in range(DT)]
                for t in range(DT):
                    for J in range(DT):
                        nc.tensor.matmul(
                            ps_s[J][:],
                            wst_sb[:, t * D + J * P:t * D + (J + 1) * P],
                            dec_sb[:, t * Bc:(t + 1) * Bc],
                            start=(t == 0), stop=(t == DT - 1))
                for J in range(DT):
                    nc.vector.tensor_copy(out=sT[:, J * Bc:(J + 1) * Bc],
                                          in_=ps_s[J][:])
            if debug_outs:
                nc.sync.dma_start(sT_dbg[:, :], sT[:])

            ps_h = es.enter_context(tc.tile_pool(name="ps_h", bufs=2, space="PSUM"))
            ps_sc = es.enter_context(tc.tile_pool(name="ps_sc", bufs=1, space="PSUM"))
            ps_cx = es.enter_context(tc.tile_pool(name="ps_cx", bufs=1, space="PSUM"))
            ps_t = es.enter_context(tc.tile_pool(name="ps_t", bufs=1, space="PSUM"))

            def emit_scores(b, c):
                eT, _ = state[(b, c)]
                # U[p, l] = sum_j v[j*128+p] * tanh_j[p, l] accumulates on the
                # DVE (per-partition scalar FMA); the PE only does the final
                # cross-partition reduction with a ones vector.
                U = uv_p.tile([P, LCHUNK], F32, tag="U", name="U")
                for j in range(DT):
                    ph = ps_h.tile([P, LCHUNK], F32, tag="ph", name="ph")
                    for t in range(ET):
                        nc.tensor.matmul(
                            ph[:],
                            whT_sb[:, t * D + j * P:t * D + (j + 1) * P],
                            eT[:, t * LCHUNK:(t + 1) * LCHUNK],
                            start=(t == 0), stop=(t == ET - 1))
                    tj = tj_p.tile([P, LCHUNK], F32, tag="tj", name="tj")
                    nc.scalar.activation(tj[:], ph[:], AF.Tanh,
                                         bias=sT[:, j * Bc + b:j * Bc + b + 1])
                    if debug_outs and b == 0 and c == 0 and j == 0:
                        nc.sync.dma_start(tj_dbg[:, :], tj[:])
                    if j == 0:
                        nc.vector.tensor_scalar_mul(U[:].bitcast(F32R), tj[:],
                                                    v32[:, 0:1])
                    else:
                        nc.vector.scalar_tensor_tensor(
                            U[:].bitcast(F32R), tj[:], v32[:, j:j + 1], U[:],
                            mybir.AluOpType.mult, mybir.AluOpType.add)
                psc = ps_sc.tile([1, LCHUNK], F32, tag="psc", name="psc")
                nc.tensor.matmul(psc[:], ones[:].bitcast(F32R),
                                 U[:].bitcast(F32R), start=True, stop=True)

                if debug_outs:
                    scrow = rows_p.tile([1, LCHUNK], F32, tag="scrow",
                                        name="scrow")
                    nc.vector.tensor_copy(out=scrow[:], in_=psc[:])
                    nc.sync.dma_start(
                        sc_dbg[b:b + 1, c * LCHUNK:(c + 1) * LCHUNK], scrow[:])

                # exp (no max subtraction; scores are bounded) + partial Z
                erow = state[("erow", b)]
                zc = rows_p.tile([1, 1], F32, tag=f"zc{c % 2}", name="zc")
                nc.scalar.activation(erow[:, c * LCHUNK:(c + 1) * LCHUNK],
                                     psc[:], AF.Exp, accum_out=zc[:])
                if c == 0:
                    z = rows_p.tile([1, 1], F32, tag=f"z{b % 2}", name="z")
                    nc.vector.tensor_copy(out=z[:], in_=zc[:])
                    state[("z", b)] = z
                else:
                    z = state[("z", b)]
                    nc.vector.tensor_add(out=z[:], in0=z[:], in1=zc[:])

                # transpose the chunk's weights to [128, LCT] f32
                wrect = rows_p.tile([LCT, P], F32, tag=f"wrect{c % 2}",
                                    name="wrect")
                nc.scalar.dma_start(wrect[:],
                                    erow[:, c * LCHUNK:(c + 1) * LCHUNK])
                ps = ps_t.tile([P, 512], F32, tag="ps_t", name="ps_wt")
                nc.tensor.transpose(ps[:, :LCT], wrect[:], ident[0:LCT, 0:LCT])
                wT = rows_p.tile([P, LCT], F32, tag=f"wT{c % 2}", name="wT")
                nc.vector.tensor_copy(out=wT[:], in_=ps[:, :LCT])
                state[("wT", b, c)] = wT

            def emit_ctx(b, c):
                """V[p, e] = sum_k w[k*128+p] * encN_k[p, e] on the DVE, then
                the PE folds partitions into the batch-long PSUM accumulators
                (one bank per 512-wide E block)."""
                _, eN = state.pop((b, c))
                wT = state.pop(("wT", b, c))
                V = uv_p.tile([P, E], F32, tag="V", name="V")
                for k in range(LCT):
                    if k == 0:
                        nc.vector.tensor_scalar_mul(
                            V[:].bitcast(F32R), eN[:, 0:E], wT[:, 0:1])
                    else:
                        nc.vector.scalar_tensor_tensor(
                            V[:].bitcast(F32R), eN[:, k * E:(k + 1) * E],
                            wT[:, k:k + 1], V[:],
                            mybir.AluOpType.mult, mybir.AluOpType.add)
                if c == 0:
                    pcs = [ps_cx.tile([1, 512], F32, tag=f"pc{n}",
                                      name=f"pc{n}") for n in range(NE)]
                    state[("pc", b)] = pcs
                else:
                    pcs = state[("pc", b)]
                for n in range(NE):
                    nc.tensor.matmul(pcs[n][:], ones[:].bitcast(F32R),
                                     V[:, n * 512:(n + 1) * 512].bitcast(F32R),
                                     start=(c == 0), stop=(c == NCH - 1))

            def emit_batch_out(b):
                erow = state.pop(("erow", b))
                z = state.pop(("z", b))
                pcs = state.pop(("pc", b))
                rz = rows_p.tile([1, 1], F32, tag=f"rz{b % 2}", name="rz")
                nc.vector.reciprocal(rz[:], z[:])
                nc.vector.tensor_scalar_mul(erow[:], erow[:], rz[:])
                nc.scalar.dma_start(attn_o[b:b + 1, :], erow[:])
                ctx_row = rows_p.tile([1, E], F32, tag=f"ctx{b % 2}",
                                      name=f"ctx{b}")
                for n in range(NE):
                    nc.vector.tensor_copy(out=ctx_row[:, n * 512:(n + 1) * 512],
                                          in_=pcs[n][:])
                nc.vector.tensor_scalar_mul(ctx_row[:], ctx_row[:], rz[:])
                nc.scalar.dma_start(ctx_o[b:b + 1, :], ctx_row[:])

            emit_load(0, 0, parts="N")

            chunks = [(b, c) for b in range(Bc) for c in range(NCH)]
            for rep in range(reps):
                for i, (b, c) in enumerate(chunks):
                    if c == 0:
                        erow = rows_p.tile([1, L], F32, tag=f"erow{b % 2}",
                                           name=f"erow{b}")
                        state[("erow", b)] = erow
                    nxt = i + 1
                    if nxt < len(chunks):
                        emit_load(*chunks[nxt])
                    elif rep + 1 < reps:
                        emit_load(*chunks[0])
                    emit_scores(b, c)
                    emit_ctx(b, c)
                    if c == NCH - 1:
                        emit_batch_out(b)

    return nc


_cache = {}


def _get_nc(reps=1):
    key = ("nc", reps)
    if key not in _cache:
        _install_waitsplit()
        _cache[key] = _build(reps=reps)
    return _cache[key]


def prepare_in_maps(inputs):
    P = _P
    B, L, D, E = _B, _L, _D, _E
    LCHUNK = _LCHUNK
    DT, ET, NCH, LCT = D // P, E // P, L // LCHUNK, LCHUNK // P
    Bc = B // _NCORES

    enc = np.asarray(inputs["enc_outputs"], dtype=np.float32)
    dec = np.asarray(inputs["dec_hidden"], dtype=np.float32)
    W_s = np.asarray(inputs["W_s"], dtype=np.float32)
    W_h = np.asarray(inputs["W_h"], dtype=np.float32)
    v = np.asarray(inputs["v"], dtype=np.float32)

    enc_bf = enc.astype(_BF16)
    # encN[b, c, p, k, e] = enc[b, c*LCHUNK + k*128 + p, e]
    encN = np.ascontiguousarray(
        enc_bf.reshape(B, NCH, LCT, P, E).transpose(0, 1, 3, 2, 4)
    ).reshape(B, NCH, P, LCT * E)
    # encT[b, c, p, t, l] = enc[b, c*LCHUNK + l, t*128 + p]
    encT = np.ascontiguousarray(
        enc_bf.reshape(B, NCH, LCHUNK, ET, P).transpose(0, 1, 4, 3, 2)
    ).reshape(B, NCH, P, ET * LCHUNK)
    # whT[p, t, d] = W_h[d, t*128 + p]
    whT = np.ascontiguousarray(
        W_h.T.astype(_BF16).reshape(ET, P, D).transpose(1, 0, 2)
    ).reshape(P, ET * D)
    # wsT[p, t, d] = W_s[d, t*128 + p]
    wsT = np.ascontiguousarray(
        W_s.T.astype(_BF16).reshape(DT, P, D).transpose(1, 0, 2)
    ).reshape(P, DT * D)
    vT = np.ascontiguousarray(v.astype(_BF16).reshape(DT, P).T)

    in_maps = []
    for i in range(_NCORES):
        dcore = dec[i * Bc:(i + 1) * Bc]
        # decT[p, t, b] = dec[b, t*128 + p]
        decT = np.ascontiguousarray(
            dcore.T.astype(_BF16).reshape(DT, P, Bc).transpose(1, 0, 2)
        ).reshape(P, DT * Bc)
        in_maps.append({
            "encN": encN[i * Bc:(i + 1) * Bc],
            "encT": encT[i * Bc:(i + 1) * Bc],
            "whT": whT,
            "wsT": wsT,
            "decT": decT,
            "vT": vT,
        })
    return in_maps


def run(inputs, trace=False, **run_kwargs):
    """Run on 8 NeuronCores; returns (ctx, attn, BassKernelResults)."""
    from concourse.bass_utils import run_bass_kernel_spmd

    nc = _get_nc()
    in_maps = prepare_in_maps(inputs)
    res = run_bass_kernel_spmd(nc, in_maps, core_ids=list(range(_NCORES)),
                               trace=trace, **run_kwargs)
    ctx = np.concatenate([res.results[i]["ctx"] for i in range(_NCORES)], axis=0)
    attn = np.concatenate([res.results[i]["attn"] for i in range(_NCORES)], axis=0)
    return ctx, attn, res


def kernel(**inputs):
    ctx, attn, _ = run(inputs, trace=False)
    return ctx, attn

